# revision 37
# baseline (speedup 1.0000x reference)
"""Causal self-attention (B=4, L=2048, C=1024, H=16) on 8 trn2 NeuronCores.

Sharding: hybrid (batch x head) tensor-parallel. Core c handles batch
b = c // 2 and heads [ (c%2)*8, (c%2)*8 + 8 ).  Each core computes a
partial projection output (L, C) for its batch; the host sums the two
partials per batch (the Wp row-sharded all-reduce done host-side).

v2 design (vs the f32r baseline):
  - all matmul operands in bf16 (f32 PSUM accumulation): halves HBM+SBUF
    traffic, enables FWL weight loads, no f32r moving>=256 constraint.
  - pair-major attention: after the qk GEMMs of head-pair 0, its entire
    st->exp->av stream is emitted, so the ACT engine's exp work overlaps
    the qk GEMMs of pairs 1..3 instead of serializing after them.
  - causal narrowing: for diagonal j-tiles at offset d, the score matmul
    only covers the valid i-suffix (width 512-128d); d=2/d=3 tiles are
    packed into one PSUM tile so one ACT instruction exps both.
  - e0/e1 head halves of each score tile live at fixed col offsets 0/512
    (separate PSUM banks) so no matmul crosses a bank boundary.
  - v-aug tiles and the output projection are interleaved into the
    attention stream as PE filler (keeps HAM un-throttled).
  - softmax denominators: ones-column trick in v_aug accumulates the
    denominator as yp row 64; both heads' rows are copied (DVE) into a
    [2,512] tile and broadcast to 128 partitions with one K=2 matmul.
  - DMA order: wq0/wk0 before x so the first GEMM starts ~2us in.
"""
import sys
import os

sys.path.insert(0, "/opt/trn_rl_repo")

import numpy as np

B, L, C, H, HD = 4, 2048, 1024, 16, 64
NCORE = 8

_compiled = {}


def _build():
    import concourse.bass as bass
    import concourse.mybir as mybir
    import concourse.tile as tile
    from concourse import bacc

    dt = mybir.dt
    f32 = dt.float32
    f32r = dt.float32r
    bf16 = dt.bfloat16
    Exp = mybir.ActivationFunctionType.Exp
    PSUM = bass.MemorySpace.PSUM

    nc = bacc.Bacc("TRN2", target_bir_lowering=False, debug=False, num_devices=NCORE)

    # x^T stored token-sliced: [lch, c-in-chunk, kt, tok] so each 512-token
    # slice is one 8KB-contiguous-per-partition DMA
    xT8 = nc.dram_tensor("xT8", [4, 128, 8, 512], bf16, kind="ExternalInput")
    wqp = nc.dram_tensor("wqp", [4, 128, 1024], bf16, kind="ExternalInput")
    wkp = nc.dram_tensor("wkp", [4, 128, 1024], bf16, kind="ExternalInput")
    wv8 = nc.dram_tensor("wv8", [8, 128, 512], bf16, kind="ExternalInput")
    wp4 = nc.dram_tensor("wp4", [4, 128, 1024], bf16, kind="ExternalInput")
    cmk = nc.dram_tensor("cmk", [128, 256], bf16, kind="ExternalInput")
    pdb = nc.dram_tensor("pdb", [128, 16], f32, kind="ExternalInput")
    zb1 = nc.dram_tensor("zb1", [128, 1], f32, kind="ExternalInput")
    onesb = nc.dram_tensor("onesb", [33, 64], bf16, kind="ExternalInput")
    one8 = nc.dram_tensor("one8", [128, 8], bf16, kind="ExternalInput")
    out = nc.dram_tensor("out", [L, C], f32, kind="ExternalOutput")

    with tile.TileContext(nc) as tc:
        with (
            tc.tile_pool(name="persist", bufs=1) as persist,
            tc.tile_pool(name="xT", bufs=8) as xp,
            tc.tile_pool(name="qkT", bufs=8) as qkp,
            tc.tile_pool(name="vaug", bufs=16) as vaugp,
            tc.tile_pool(name="yT", bufs=4) as ytp,
            tc.tile_pool(name="P", bufs=12) as Pp,
            tc.tile_pool(name="sd", bufs=4) as sdp,
            tc.tile_pool(name="rr", bufs=4) as rrp,
            tc.tile_pool(name="osb", bufs=2) as osbp,
            tc.tile_pool(name="wq", bufs=4) as wqp_p,
            tc.tile_pool(name="wk", bufs=4) as wkp_p,
            tc.tile_pool(name="wv", bufs=8) as wvp,
            tc.tile_pool(name="wp", bufs=4) as wpp,
            tc.tile_pool(name="gp", bufs=2, space=PSUM) as gp,
            tc.tile_pool(name="stps", bufs=2, space=PSUM) as stps,
            tc.tile_pool(name="yps", bufs=2, space=PSUM) as yps,
        ):
            # ---- weight + x DMAs in latency order, split across the two
            # hardware DGE queues (Sync + Activation engines) ----
            wqts, wkts = [], []
            for p in range(4):
                wqts.append(wqp_p.tile([128, 1024], bf16, tag="wq", name=f"wq{p}"))
                wkts.append(wkp_p.tile([128, 1024], bf16, tag="wk", name=f"wk{p}"))
            # x loads are token-major (one 512-token slice per DMA, 8KB
            # contiguous per partition) so ch-0 GEMMs and band-0 attention
            # start after ~1MB of x instead of the full 4MB.
            xls = [
                xp.tile([128, 8, 512], bf16, tag="x", name=f"x{lch}")
                for lch in range(4)
            ]
            wvts = [
                wvp.tile([128, 512], bf16, tag="wv", name=f"wv{kt}") for kt in range(8)
            ]
            # critical path on sync queue: wq0, x0 (in two halves), x1..x3
            nc.sync.dma_start(wqts[0][:], wqp[0])
            nc.sync.dma_start(xls[0][:, 0:4, :], xT8[0][:, 0:4, :])
            nc.sync.dma_start(xls[0][:, 4:8, :], xT8[0][:, 4:8, :])
            for lch in range(1, 4):
                nc.sync.dma_start(xls[lch][:], xT8[lch])
            # parallel queue (scalar engine DGE): consts (tiny, needed by
            # the first exp/va DVE ops - keep them off every wait path),
            # then wk0, wv, rest
            cm = persist.tile([128, 256], bf16, tag="cm")
            nc.scalar.dma_start(cm[:], cmk[:])
            pb = persist.tile([128, 16], f32, tag="pb")
            nc.scalar.dma_start(pb[:], pdb[:])
            zb = persist.tile([128, 1], f32, tag="zb")
            nc.scalar.dma_start(zb[:], zb1[:])
            onb = persist.tile([33, 64], bf16, tag="onb")
            nc.scalar.dma_start(onb[:], onesb[:])
            ones8 = persist.tile([128, 8], bf16, tag="ones8")
            nc.scalar.dma_start(ones8[:], one8[:])
            nc.scalar.dma_start(wkts[0][:], wkp[0])
            for kt in range(8):
                nc.scalar.dma_start(wvts[kt][:], wv8[kt])
            for p in range(1, 4):
                nc.scalar.dma_start(wqts[p][:], wqp[p])
                nc.scalar.dma_start(wkts[p][:], wkp[p])
            wpts = []
            for kt4 in range(4):
                t = wpp.tile([128, 1024], bf16, tag="wp", name=f"wp{kt4}")
                nc.scalar.dma_start(t[:], wp4[kt4])
                wpts.append(t)

            yts = [ytp.tile([128, L], bf16, tag="yt", name=f"yt{i}") for i in range(4)]
            qts, kts = [], []
            vats = {}

            def prep_qk(p):
                """Allocate qT/kT for pair p; return the 8 accumulation-group
                emitters (2 W x 4 ch) to be spread as PE filler."""
                qT = qkp.tile([128, L], bf16, tag="qk", name=f"qT{p}")
                kT = qkp.tile([128, L], bf16, tag="qk", name=f"kT{p}")
                qts.append(qT)
                kts.append(kT)
                chunks = []
                for ch in range(4):
                    for W, dst in ((wqts[p], qT), (wkts[p], kT)):
                        def go(W=W, dst=dst, ch=ch):
                            ps = gp.tile([128, 512], f32, tag="g", name="qkps")
                            for kt in range(8):
                                nc.tensor.matmul(
                                    ps[:],
                                    W[:, kt * 128 : (kt + 1) * 128],
                                    xls[ch][:, kt, :],
                                    start=(kt == 0),
                                    stop=(kt == 7),
                                )
                            nc.vector.tensor_copy(
                                dst[:, ch * 512 : (ch + 1) * 512], ps[:]
                            )
                        chunks.append(go)
                return chunks

            def emit_va(jt):
                ps = gp.tile([128, 512], f32, tag="g", name="vps")
                to = (jt % 4) * 128
                for kt in range(8):
                    nc.tensor.matmul(
                        ps[:],
                        xls[jt // 4][:, kt, to : to + 128],
                        wvts[kt][:],
                        start=(kt == 0),
                        stop=(kt == 7),
                    )
                va = vaugp.tile([128, 8, 65], bf16, tag="va", name=f"va{jt}")
                nc.vector.tensor_scalar_mul(
                    va[:, :, 0:64],
                    ps[:].rearrange("p (h e) -> p h e", e=64),
                    pb[:, jt : jt + 1],
                )
                nc.vector.tensor_scalar_mul(
                    va[:, :, 64:65],
                    ones8[:].rearrange("p (h o) -> p h o", o=1),
                    pb[:, jt : jt + 1],
                )
                vats[jt] = va

            pending_norm = [None]

            def flush_norm():
                if pending_norm[0] is None:
                    return
                p, it, yp, sd = pending_norm[0]
                pending_norm[0] = None
                # two K=1 broadcasts in concurrent PE row groups (0 and 32)
                for e in (0, 1):
                    rb = gp.tile([64, 512], f32, tag="g", name=f"rb{e}")
                    nc.tensor.matmul(
                        rb[:],
                        onb[32 * e : 32 * e + 1, :],
                        sd[32 * e : 32 * e + 1, :],
                        start=True,
                        stop=True,
                    )
                    rr = rrp.tile([64, 512], f32, tag="rr")
                    nc.vector.reciprocal_approx_fast(rr[:], rb[:])
                    nc.vector.tensor_mul(
                        yts[p][e * 64 : (e + 1) * 64, it * 512 : (it + 1) * 512],
                        yp[e][0:64, :],
                        rr[:],
                    )

            def emit_proj_ic(ic):
                osb = osbp.tile([128, 1024], f32, tag="o")
                for ch in range(2):
                    ps = gp.tile([128, 512], f32, tag="g", name="pj")
                    for kt4 in range(4):
                        nc.tensor.matmul(
                            ps[:],
                            yts[kt4][:, ic * 128 : (ic + 1) * 128],
                            wpts[kt4][:, ch * 512 : (ch + 1) * 512],
                            start=(kt4 == 0),
                            stop=(kt4 == 3),
                        )
                    nc.vector.tensor_copy(osb[:, ch * 512 : (ch + 1) * 512], ps[:])
                nc.sync.dma_start(out[ic * 128 : (ic + 1) * 128, :], osb[:])

            def attn_pair(p, fill_q, fill_stride):
                qT, kT = qts[p], kts[p]
                hook_n = [0]
                for it in range(4):
                    nj = 4 * it + 4
                    # av_parts[jt] = (e0 slice, e1 slice, band col off)
                    av_parts = {}
                    gidx = [0]

                    def group_hook():
                        # interleave deferred work at score-group boundaries;
                        # filler work runs at low scheduler priority so it
                        # never delays the st->exp feed
                        if gidx[0] == 0:
                            flush_norm()
                        if p == 3 and it > 0 and gidx[0] < 4:
                            emit_proj_ic(4 * (it - 1) + gidx[0])
                        npop = 0
                        if fill_q:
                            if p == 0:
                                npop = 2 if hook_n[0] < 2 else 1
                            elif hook_n[0] % fill_stride == 0:
                                npop = 1
                        for _ in range(min(npop, len(fill_q))):
                            fill_q.pop(0)()
                        gidx[0] += 1
                        hook_n[0] += 1

                    def emit_st(jts):
                        # jts: list of (jt, coloff, w); packed into one PSUM
                        # tile with e0 in bank cols [0:512), e1 in [512:1024)
                        stp = stps.tile([128, 1024], f32, tag="st", name="stp")
                        P = Pp.tile([128, 1024], bf16, tag="P", name="P")
                        tot = sum(w for _, _, w in jts)
                        for jt, co, w in jts:
                            d = jt - 4 * it
                            off = 128 * d if d > 0 else 0
                            for e in (0, 1):
                                nc.tensor.matmul(
                                    stp[:, 512 * e + co : 512 * e + co + w],
                                    kT[64 * e : 64 * e + 64, jt * 128 : (jt + 1) * 128],
                                    qT[64 * e : 64 * e + 64, it * 512 + off : (it + 1) * 512],
                                    start=True,
                                    stop=True,
                                )
                        if tot == 512:
                            nc.scalar.activation(
                                P[:], stp[:], Exp, scale=0.125, bias=zb[:, 0:1]
                            )
                        else:
                            pv = P[:].rearrange("p (h i) -> p h i", i=512)[:, :, 0:tot]
                            sv = stp[:].rearrange("p (h i) -> p h i", i=512)[:, :, 0:tot]
                            nc.scalar.activation(
                                pv, sv, Exp, scale=0.125, bias=zb[:, 0:1]
                            )
                        for jt, co, w in jts:
                            d = jt - 4 * it
                            if d >= 0:
                                # first 128 cols of this jt's slice hold the
                                # partial triangle; multiply by causal mask
                                mv = P[:].rearrange("p (h i) -> p h i", i=512)[
                                    :, :, co : co + 128
                                ]
                                cv = cm[:].rearrange("p (h i) -> p h i", i=128)
                                nc.vector.tensor_mul(mv, mv, cv)
                            off = 128 * d if d > 0 else 0
                            av_parts[jt] = (
                                P[:, co : co + w],
                                P[:, 512 + co : 512 + co + w],
                                off,
                            )
                        group_hook()

                    # off-diagonal full tiles, then d0, d1, then d2+d3 packed
                    for jt in range(4 * it):
                        emit_st([(jt, 0, 512)])
                    emit_st([(4 * it, 0, 512)])
                    emit_st([(4 * it + 1, 0, 384)])
                    emit_st([(4 * it + 2, 0, 256), (4 * it + 3, 256, 128)])

                    yp = [
                        yps.tile([65, 512], f32, tag="y", name=f"yp{p}_{it}_{e}")
                        for e in (0, 1)
                    ]
                    for jt in range(nj):
                        p0, p1, off = av_parts[jt]
                        for e, Pslice in ((0, p0), (1, p1)):
                            nc.tensor.matmul(
                                yp[e][:, off:512],
                                vats[jt][:, 2 * p + e, :],
                                Pslice,
                                start=(jt == 0),
                                stop=(jt == nj - 1),
                            )
                    sd = sdp.tile([33, 512], bf16, tag="sd")
                    for e in (0, 1):
                        nc.vector.tensor_copy(
                            sd[32 * e : 32 * e + 1, :], yp[e][64:65, :]
                        )
                    pending_norm[0] = (p, it, yp, sd)

            all_chunks = [prep_qk(p) for p in range(4)]
            # only the ch-0 q/k chunks upfront: band 0 depends on just these
            for c in all_chunks[0][:2]:
                c()
            # pair 0 fillers: band-0's va tiles first (popped 2-per-hook so
            # they don't delay the first st->exp), then pair 0's remaining
            # q/k chunks (band it needs ch it), remaining va, pair 1's qk
            def mkva(jt):
                return lambda: emit_va(jt)

            c0 = all_chunks[0]
            fq0 = [mkva(0), mkva(1), mkva(2), mkva(3)]
            fq0 += [c0[2], c0[3], mkva(4), mkva(5), mkva(6), mkva(7)]
            fq0 += [c0[4], c0[5], mkva(8), mkva(9), mkva(10), mkva(11)]
            fq0 += [c0[6], c0[7], mkva(12), mkva(13), mkva(14), mkva(15)]
            fq0 += all_chunks[1]
            attn_pair(0, fq0, 1)
            attn_pair(1, list(all_chunks[2]), 2)
            attn_pair(2, list(all_chunks[3]), 2)
            attn_pair(3, [], 1)
            flush_norm()
            for ic in range(12, 16):
                emit_proj_ic(ic)

    nc.compile()
    return nc


def _get_nc():
    if "v2" not in _compiled:
        _compiled["v2"] = _build()
    return _compiled["v2"]


def _prep_inputs(x, Wq, Wk, Wv, Wp, attn_mask):
    import ml_dtypes

    bf16 = ml_dtypes.bfloat16

    x = np.ascontiguousarray(np.asarray(x, np.float32))
    Wq = np.asarray(Wq, np.float32)
    Wk = np.asarray(Wk, np.float32)
    Wv = np.asarray(Wv, np.float32)
    Wp = np.asarray(Wp, np.float32)
    am = np.asarray(attn_mask)

    # causal multiplicative triangle mask (duplicated for the 2 heads of a
    # pair); applied to the first 128 columns of each diagonal tile slice
    rr = np.arange(128)[:, None]
    ii = np.arange(128)[None, :]
    tri = (rr <= ii).astype(np.float32)
    cmask = np.ascontiguousarray(np.tile(tri, (1, 2)).astype(bf16))

    onesb = np.ones((33, 64), bf16)

    halves = []
    for hh in range(2):
        WqT = Wq[hh * 512 : (hh + 1) * 512, :].T  # (C, 512)
        WkT = Wk[hh * 512 : (hh + 1) * 512, :].T
        WvT = Wv[hh * 512 : (hh + 1) * 512, :].T
        WpT = Wp[:, hh * 512 : (hh + 1) * 512].T  # (512, C)
        wqp = np.ascontiguousarray(
            WqT.reshape(8, 128, 4, 128).transpose(2, 1, 0, 3).reshape(4, 128, 1024)
        ).astype(bf16)
        wkp = np.ascontiguousarray(
            WkT.reshape(8, 128, 4, 128).transpose(2, 1, 0, 3).reshape(4, 128, 1024)
        ).astype(bf16)
        wv8 = np.ascontiguousarray(WvT.reshape(8, 128, 512)).astype(bf16)
        wp4 = np.ascontiguousarray(WpT.reshape(4, 128, 1024)).astype(bf16)
        halves.append((wqp, wkp, wv8, wp4))

    in_maps = []
    for c in range(NCORE):
        b, hh = c // 2, c % 2
        # [lch, c-in-chunk, kt, tok]: each (lch) block is one DMA with 8KB
        # contiguous per partition row
        xT = np.ascontiguousarray(
            x[b].T.reshape(8, 128, 4, 512).transpose(2, 1, 0, 3)
        ).astype(bf16)
        padb = (am[b].reshape(16, 128).T != 0).astype(np.float32)
        wqp, wkp, wv8, wp4 = halves[hh]
        in_maps.append(
            {
                "xT8": xT,
                "wqp": wqp,
                "wkp": wkp,
                "wv8": wv8,
                "wp4": wp4,
                "cmk": cmask,
                "one8": np.ones((128, 8), bf16),
                "zb1": np.zeros((128, 1), np.float32),
                "onesb": onesb,
                "pdb": np.ascontiguousarray(padb),
            }
        )
    return in_maps


def _run(in_maps, trace=False, tmpdir=None):
    from concourse.bass_utils import run_bass_kernel_spmd

    nc = _get_nc()
    if trace:
        _register_ntff_hook()
    return run_bass_kernel_spmd(
        nc, in_maps, list(range(NCORE)), trace=trace, tmpdir=tmpdir
    )


def _register_ntff_hook():
    """The agent image's antenv lacks axon_hooks; register the NTFF
    profiling hook manually so trace=True yields exec_time_ns."""
    import types
    import antenv

    if "antenv.axon_hooks" in sys.modules:
        return
    mod = types.ModuleType("antenv.axon_hooks")
    hook = [None]
    mod.set_axon_ntff_profile_hook = lambda h: hook.__setitem__(0, h)
    mod.get_axon_ntff_profile_hook = lambda: hook[0]
    sys.modules["antenv.axon_hooks"] = mod
    antenv.axon_hooks = mod
    if "/root/.axon_site" not in sys.path:
        sys.path.insert(0, "/root/.axon_site")
    from trn_agent_boot.trn_boot import _ntff_profile_via_ctypes

    mod.set_axon_ntff_profile_hook(
        _ntff_profile_via_ctypes("/opt/axon/libaxon_pjrt.so")
    )


def kernel(x, Wq, Wk, Wv, Wp, attn_mask):
    in_maps = _prep_inputs(x, Wq, Wk, Wv, Wp, attn_mask)
    res = _run(in_maps)
    y = np.empty((B, L, C), np.float32)
    for b in range(B):
        y[b] = res.results[2 * b]["out"] + res.results[2 * b + 1]["out"]
    return y
